# revision 53
# baseline (speedup 1.0000x reference)
"""Trainium2 Bass kernel for nn_DCTHighPass.

Reference computation (per sample, 512x512 RGB image):
  gray = 0.299 R + 0.587 G + 0.114 B
  tiles = 8x8 blocks of gray (64x64 tiles, row-major (ti, tj))
  mag = |fft2(tile)|
  (buggy mask touches only batch 3:6 / fft rows 3:6 -> never sampled below)
  img = mag tiles stacked into [4096*8, 8]
  out = bilinear_resize(img, 512, 512)

Key reduction: the height downsample (32768 -> 512, factor 64) samples only
input rows 64i+31 and 64i+32 with weight 0.5 each, i.e. fft-row 7 of tile
(ti=i//8, tj=8*(i%8)+3) and fft-row 0 of tile (ti, tj=8*(i%8)+4).  fft rows
0/7 of an 8x8 real tile need only three 8-weight row reductions of the tile
(plain sum, cos, sin), followed by an 8-point DFT along columns.  The width
upsample (8 -> 512) is a fixed [8,512] matrix.  So per output row i:
  v = 0.5*(|DFT(cos/sin rowsums of tile tj3)| + |DFT(colsum of tile tj4)|)
  out[i, :] = v @ W8
Only image columns 64p+24 .. 64p+39 (p = i%8) are ever used.

Layout strategy (the v1 kernel was DMA-bound at 75% on strided 64B
descriptors; this version is ~2.8x faster):
  - host gathers the 128 needed columns into a dense [B, 128p, 3ch*4q*128c]
    bf16 array, so device loads are contiguous 3KB-per-partition DMAs
    (strided 64B descriptors ran at ~9B/ns/engine vs 22.5 peak);
  - all matmuls and constants run in bf16 (fp32 pays 4 cycles/row on PE,
    bf16 pays 1; PSUM still accumulates fp32);
  - output is written bf16 (halves store traffic), merged 4KB-contiguous
    stores; host upcasts to fp32 (rel err ~0.5% vs the 2e-2 gate);
  - all 4 sample-pairs are emitted interleaved stage-by-stage
    (CFG interleave=4) so the Tile scheduler overlaps pairs.

Pipeline per 2 samples (per core, batch of 8 samples):
  load   xin [128, 1536] bf16 per sample (gpsimd queue)
  stage1 (PE): per (smp, q, ch): x chunk stationary, gray-coef-scaled
         wred [128,48] moving, ch-accumulated in PSUM
         -> ps1 [128=(p,cc), (q: type,tI_l)] -> rhs2 bf16 (DVE copy)
  stage2 (PE): 6 block-diag DFT matmuls -> psQ [64=(p,k), 512]
  magnitude (ACT/DVE): sq -> pair-sum -> 0.5*sqrt -> vt [64, 128] bf16
  stage3 (PE): vt stationary @ p-masked replicated W8 -> 8x [128, 512]
  drain PSUM -> out_all [128=(smp,tI), 8p*512j] bf16 (DVE/ACT copies)
  store ys rows 8*tI+p: two [64, 2048] DMAs per sample (sync queue)
"""

import sys

sys.path.insert(0, "/opt/trn_rl_repo")

import numpy as np
import ml_dtypes

from concourse import bacc
import concourse.mybir as mybir
from concourse.tile import TileContext
from concourse.bass_utils import run_bass_kernel_spmd

N_CORES = 8
B_FULL = 64
B_CORE = B_FULL // N_CORES  # 8 samples per core
H = W = 512
K = 8  # fft tile size
NQ = 4  # 128-row chunks per image
F32 = mybir.dt.float32
BF16 = mybir.dt.bfloat16
F32R = mybir.dt.float32r

# image columns ever sampled by the width resize: 64p+24 .. 64p+39
_COLS = np.concatenate([np.arange(64 * p + 24, 64 * p + 40) for p in range(K)])


# ----------------------------------------------------------------------------
# host-side constants
# ----------------------------------------------------------------------------
def _make_constants():
    j = np.arange(K)
    cosr = np.cos(2 * np.pi * j / K)
    sinr = np.sin(2 * np.pi * j / K)

    # wred [128, 144]: free = 48*ch + 16*type + tI_l, gray coef folded in;
    # type 0: plain sum (A), 1: cos rowsum (Cr), 2: sin rowsum (Ci);
    # partition = 8*tI_l + row
    coef = [0.299, 0.587, 0.114]
    wtypes = [np.ones(K), cosr, sinr]
    wred = np.zeros((128, 144), dtype=np.float32)
    for ch in range(3):
        for ty in range(3):
            for t in range(16):
                wred[8 * t : 8 * t + 8, 48 * ch + 16 * ty + t] = (
                    coef[ch] * wtypes[ty]
                )

    # dft matrices C[v,c] = cos(2pi v c/8), S[v,c] = sin(2pi v c/8)
    v = np.arange(K)
    C8 = np.cos(2 * np.pi * np.outer(v, j) / K).astype(np.float32)
    S8 = np.sin(2 * np.pi * np.outer(v, j) / K).astype(np.float32)

    # dftc [128, 320]: 5 blocks of 64 cols: [C*g0 | S*g0 | -S*g0 | C*g1 | S*g1]
    # partition = 16p + cc (cc in 0..15, g = cc//8); out col = 64*s + 8p + k
    dftc = np.zeros((128, 320), dtype=np.float32)
    for p in range(8):
        for cc in range(16):
            g, c = divmod(cc, 8)
            for k in range(8):
                cv, sv = C8[k, c], S8[k, c]
                if g == 0:
                    dftc[16 * p + cc, 0 + 8 * p + k] = cv
                    dftc[16 * p + cc, 64 + 8 * p + k] = sv
                    dftc[16 * p + cc, 128 + 8 * p + k] = -sv
                else:
                    dftc[16 * p + cc, 192 + 8 * p + k] = cv
                    dftc[16 * p + cc, 256 + 8 * p + k] = sv

    # W8 [8, 512]: bilinear width resize 8 -> 512 (align_corners=False)
    src = (np.arange(W) + 0.5) * (K / W) - 0.5
    src = np.clip(src, 0.0, K - 1.0)
    i0 = np.floor(src).astype(np.int64)
    i1 = np.minimum(i0 + 1, K - 1)
    fr = (src - i0).astype(np.float32)
    W8 = np.zeros((K, W), dtype=np.float32)
    for jj in range(W):
        W8[i0[jj], jj] += 1.0 - fr[jj]
        W8[i1[jj], jj] += fr[jj]

    # wrep [64, 8*512]: block p holds W8 on partitions 8p..8p+7, zero elsewhere
    wrep = np.zeros((64, 8 * W), dtype=np.float32)
    for p in range(8):
        wrep[8 * p : 8 * p + 8, W * p : W * p + W] = W8

    return (
        wred.astype(ml_dtypes.bfloat16),
        dftc.astype(ml_dtypes.bfloat16),
        wrep.astype(ml_dtypes.bfloat16),
    )


_WRED, _DFTC, _WREP = _make_constants()


# ----------------------------------------------------------------------------
# bass program (identical on all cores; per-core inputs differ)
# ----------------------------------------------------------------------------
CFG = dict(xin_bufs=4, mid_bufs=5, out_bufs=3, ps1_bufs=3, ps2_bufs=2,
           ps3_bufs=3, copy_pat="avavavav", split_load=1, split_store=2, interleave=4, direct_store=False, load_q="gpsimd", sq_on_dve=False, store_split_q="smp", s2_reorder=False)


def _build_program(repeat=1, variant="full", unroll=False):
    nold = variant in ("nold", "nodma")
    nost = variant in ("nost", "nodma")
    nc = bacc.Bacc()

    xs = nc.declare_dram_parameter("xs", [B_CORE, 128, 3 * NQ * 128], BF16, isOutput=False)
    wred_d = nc.declare_dram_parameter("wred", [128, 144], BF16, isOutput=False)
    dftc_d = nc.declare_dram_parameter("dftc", [128, 320], BF16, isOutput=False)
    wrep_d = nc.declare_dram_parameter("wrep", [64, 8 * W], BF16, isOutput=False)
    ys = nc.declare_dram_parameter("ys", [B_CORE, 1, H, W], BF16, isOutput=True)

    with TileContext(nc) as tc:
        with (
            tc.tile_pool(name="consts", bufs=1) as cpool,
            tc.tile_pool(name="xin", bufs=CFG["xin_bufs"]) as xpool,
            tc.tile_pool(name="mid", bufs=CFG["mid_bufs"]) as mpool,
            tc.tile_pool(name="outp", bufs=CFG["out_bufs"]) as opool,
            tc.tile_pool(name="ps1", bufs=CFG["ps1_bufs"], space="PSUM") as ps1pool,
            tc.tile_pool(name="ps2", bufs=CFG["ps2_bufs"], space="PSUM") as ps2pool,
            tc.tile_pool(name="ps3", bufs=CFG["ps3_bufs"], space="PSUM") as ps3pool,
        ):
            wred_sb = cpool.tile([128, 144], BF16, tag="wred")
            nc.sync.dma_start(wred_sb[:], wred_d[:])
            dftc_sb = cpool.tile([128, 320], BF16, tag="dftc")
            nc.sync.dma_start(dftc_sb[:], dftc_d[:])
            wrep_sb = cpool.tile([64, 8 * W], BF16, tag="wrep")
            nc.scalar.dma_start(wrep_sb[:], wrep_d[:])
            xconst = []
            if nold:
                # ablation: inputs loaded once, loop reads static tiles
                for smp in range(2):
                    xc = cpool.tile([128, 3 * NQ * 128], BF16, tag=f"xc{smp}")
                    nc.gpsimd.dma_start(xc[:], xs[smp])
                    xconst.append(
                        xc.rearrange("p (ch q c) -> p ch q c", ch=3, q=NQ)
                    )

            C0 = dftc_sb[:, 0:64]
            S0 = dftc_sb[:, 64:128]
            S0n = dftc_sb[:, 128:192]
            C1 = dftc_sb[:, 192:256]
            S1 = dftc_sb[:, 256:320]

            def do_loads(bg2):
                # per sample: one contiguous [128, 1536] bf16 DMA, or one
                # [128, 512] DMA per channel into its own tile (ch_tiles)
                # so stage-1 starts after the first channel lands
                if nold:
                    return lambda smp, ch, q: xconst[smp][:, ch, q]
                xn = []
                for smp in range(2):
                    bg = 2 * bg2 + smp
                    ldq = getattr(nc, CFG["load_q"])
                    if CFG.get("ch_tiles"):
                        xsv = xs[bg].rearrange("p (ch r) -> p ch r", ch=3)
                        chts = []
                        for ch in range(3):
                            t = xpool.tile(
                                [128, NQ * 128], BF16, tag=f"xc{smp}{ch}"
                            )
                            ldq.dma_start(t[:], xsv[:, ch])
                            chts.append(t.rearrange("p (q c) -> p q c", q=NQ))
                        xn.append(chts)
                    else:
                        xin = xpool.tile(
                            [128, 3 * NQ * 128], BF16, tag=f"xn{smp}"
                        )
                        ldq.dma_start(xin[:], xs[bg])
                        xn.append(
                            xin.rearrange("p (ch q c) -> p ch q c", ch=3, q=NQ)
                        )
                if CFG.get("ch_tiles"):
                    return lambda smp, ch, q: xn[smp][ch][:, q]
                return lambda smp, ch, q: xn[smp][:, ch, q]

            def do_stage1(xn):
                # gray folded into channel-accumulated row reductions
                rhs2 = mpool.tile([128, 2 * 192], BF16, tag="rhs2")
                for smp in range(2):
                    ps1 = ps1pool.tile([128, 192], F32, tag="ps1")
                    for q in range(NQ):
                        for ch in range(3):
                            lhs = (
                                xn(0, 0, 0)
                                if variant == "ld1"
                                else xn(smp, ch, q)
                            )
                            nc.tensor.matmul(
                                ps1[:, 48 * q : 48 * q + 48],
                                lhs,
                                wred_sb[:, 48 * ch : 48 * ch + 48],
                                start=(ch == 0), stop=(ch == 2),
                            )
                    nc.vector.tensor_copy(
                        rhs2[:, 192 * smp : 192 * smp + 192], ps1[:]
                    )
                return rhs2

            def do_stage2(rhs2):
                # DFT + height-blend fused via PSUM accumulation;
                # psQ [64=(p,k), 512] = [R3 | I3 | R4 | I4] of (smp,q,tI_l)
                rhs2v = rhs2.rearrange("p (s q blk) -> p s q blk", s=2, q=NQ)
                selA = rhs2v[:, :, :, 0:16]
                selCr = rhs2v[:, :, :, 16:32]
                selCi = rhs2v[:, :, :, 32:48]
                psQ = ps2pool.tile([64, 512], F32, tag="psQ")
                if CFG.get("s2_reorder"):
                    # C0 used by two consecutive matmuls -> one Ldweights
                    nc.tensor.matmul(psQ[:, 0:128], C0, selCr, start=True, stop=False)
                    nc.tensor.matmul(psQ[:, 128:256], C0, selCi, start=True, stop=False)
                    nc.tensor.matmul(psQ[:, 0:128], S0, selCi, start=False, stop=True)
                    nc.tensor.matmul(psQ[:, 128:256], S0n, selCr, start=False, stop=True)
                else:
                    nc.tensor.matmul(psQ[:, 0:128], C0, selCr, start=True, stop=False)
                    nc.tensor.matmul(psQ[:, 0:128], S0, selCi, start=False, stop=True)
                    nc.tensor.matmul(psQ[:, 128:256], C0, selCi, start=True, stop=False)
                    nc.tensor.matmul(psQ[:, 128:256], S0n, selCr, start=False, stop=True)
                nc.tensor.matmul(psQ[:, 256:384], C1, selA, start=True, stop=True)
                nc.tensor.matmul(psQ[:, 384:512], S1, selA, start=True, stop=True)
                return psQ

            def do_mag(psQ):
                # m = 0.5*sqrt(re^2 + im^2); pair-sum via strided APs
                sq = mpool.tile([64, 512], F32, tag="sq")
                sqv = sq.rearrange("p (a b c) -> p a b c", a=2, b=2)
                s34 = mpool.tile([64, 256], F32, tag="s34")
                s34v = s34.rearrange("p (a c) -> p a c", a=2)
                if CFG.get("split_mag"):
                    # per-group squares start as soon as that group's
                    # stage-2 matmuls land (R3I3 after 4, R4I4 after 6)
                    nc.scalar.activation(
                        sq[:, 0:256], psQ[:, 0:256],
                        mybir.ActivationFunctionType.Square,
                    )
                    nc.vector.tensor_add(
                        s34[:, 0:128], sqv[:, 0, 0], sqv[:, 0, 1]
                    )
                    nc.scalar.activation(
                        sq[:, 256:512], psQ[:, 256:512],
                        mybir.ActivationFunctionType.Square,
                    )
                    nc.vector.tensor_add(
                        s34[:, 128:256], sqv[:, 1, 0], sqv[:, 1, 1]
                    )
                else:
                    nc.scalar.activation(
                        sq[:], psQ[:], mybir.ActivationFunctionType.Square
                    )
                    nc.vector.tensor_add(s34v[:], sqv[:, :, 0], sqv[:, :, 1])
                m34 = mpool.tile([64, 256], F32, tag="m34")
                nc.scalar.activation(
                    m34[:], s34[:], mybir.ActivationFunctionType.Sqrt, scale=0.25
                )
                vt = mpool.tile([64, 128], BF16, tag="vt")
                nc.vector.tensor_add(vt[:], m34[:, 0:128], m34[:, 128:256])
                return vt

            def do_stage3(vt):
                # width resize; out partitions = (smp, tI)
                out_all = opool.tile([128, 8 * W], BF16, tag="out_all")
                wp = CFG.get("ps3_width", W)
                for i, p0 in enumerate(range(0, 8 * W, wp)):
                    ps3 = ps3pool.tile([128, wp], F32, tag="ps3")
                    nc.tensor.matmul(
                        ps3[:],
                        vt[:],
                        wrep_sb[:, p0 : p0 + wp],
                        start=True, stop=True,
                    )
                    dst = out_all[:, p0 : p0 + wp]
                    c = CFG["copy_pat"][i % len(CFG["copy_pat"])]
                    if c == "v":
                        nc.vector.tensor_copy(dst, ps3[:])
                    else:
                        nc.scalar.copy(dst, ps3[:])
                return out_all

            def do_stage3_direct(bg2, vt):
                # width resize with bf16 PSUM output, stored straight from
                # PSUM to HBM (no drain copies); out partitions = (smp, tI)
                if nost:
                    ysv = None
                else:
                    ysv = ys[2 * bg2 : 2 * bg2 + 2, 0].rearrange(
                        "s (t p) j -> (s t) (p j)", t=64
                    )
                for p in range(8):
                    ps3 = ps3pool.tile([128, W], BF16, tag="ps3")
                    nc.tensor.matmul(
                        ps3[:],
                        vt[:],
                        wrep_sb[:, W * p : W * p + W],
                        start=True, stop=True,
                    )
                    if ysv is not None:
                        nc.gpsimd.dma_start(ysv[:, W * p : W * p + W], ps3[:])

            def do_stores(bg2, out_all):
                # rows 8*tI + p are contiguous in (p j); split halves fire
                # as soon as their copies land
                if nost:
                    return
                ss = CFG["split_store"]
                for smp in range(2):
                    bg = 2 * bg2 + smp
                    dst = ys[bg, 0].rearrange(
                        "(t h p) j -> h t (p j)", t=64, h=ss
                    )
                    src = out_all[64 * smp : 64 * smp + 64, :].rearrange(
                        "t (h pj) -> t h pj", h=ss
                    )
                    for hh in range(ss):
                        mode = CFG.get("store_split_q")
                        if mode == "smp" and smp == 1:
                            stq = nc.gpsimd
                        elif mode == "hh" and hh % 2 == 1:
                            stq = nc.gpsimd
                        else:
                            stq = nc.sync
                        stq.dma_start(dst[hh], src[:, hh])

            rep_ctx = tc.For_i(0, repeat, 1) if repeat > 1 and not unroll else None
            if rep_ctx is not None:
                rep_ctx.__enter__()
            n_unroll = repeat if unroll else 1
            G = CFG["interleave"]
            NB = B_CORE // 2
            def emit_tail(b, vts):
                if CFG["direct_store"]:
                    do_stage3_direct(b, vts[b])
                else:
                    do_stores(b, do_stage3(vts[b]))

            skew = CFG.get("skew", 0)
            for u in range(n_unroll):
                if skew:
                    # software-pipelined emission: stage3/stores of pair
                    # b-skew follow stage1/2/mag of pair b, so stores spread
                    # across the body instead of bunching at the tail
                    xns = {b: do_loads(b) for b in range(NB)}
                    vts = {}
                    for b in range(NB):
                        vts[b] = do_mag(do_stage2(do_stage1(xns[b])))
                        if b >= skew:
                            emit_tail(b - skew, vts)
                    for b in range(NB - skew, NB):
                        emit_tail(b, vts)
                else:
                    for base in range(0, NB, G):
                        prs = list(range(base, min(base + G, NB)))
                        xns = {b: do_loads(b) for b in prs}
                        rhs = {b: do_stage1(xns[b]) for b in prs}
                        psq = {b: do_stage2(rhs[b]) for b in prs}
                        vts = {b: do_mag(psq[b]) for b in prs}
                        if CFG["direct_store"]:
                            for b in prs:
                                do_stage3_direct(b, vts[b])
                        else:
                            outs = {b: do_stage3(vts[b]) for b in prs}
                            for b in prs:
                                do_stores(b, outs[b])

            if rep_ctx is not None:
                rep_ctx.__exit__(None, None, None)

    nc.compile()
    return nc


_NC = None


def _get_program():
    global _NC
    if _NC is None:
        _NC = _build_program()
    return _NC


def _prep_inputs(x: np.ndarray) -> np.ndarray:
    """[64,3,512,512] f32 -> [64, 128, 1536] bf16 with the needed columns
    gathered and rows regrouped: out[s, p, (ch,q,c)] = x[s, ch, 128q+p, COLS[c]]."""
    xsel = x[:, :, :, _COLS]  # [64, 3, 512, 128]
    xr = xsel.reshape(B_FULL, 3, NQ, 128, 128).transpose(0, 3, 1, 2, 4)
    return np.ascontiguousarray(xr).reshape(B_FULL, 128, 3 * NQ * 128).astype(
        ml_dtypes.bfloat16
    )


def kernel(x: np.ndarray) -> np.ndarray:
    assert x.shape == (B_FULL, 3, H, W), x.shape
    x = np.ascontiguousarray(x, dtype=np.float32)
    xp = _prep_inputs(x)
    nc = _get_program()
    in_maps = []
    for c in range(N_CORES):
        in_maps.append(
            {
                "xs": xp[c * B_CORE : (c + 1) * B_CORE],
                "wred": _WRED,
                "dftc": _DFTC,
                "wrep": _WREP,
            }
        )
    res = run_bass_kernel_spmd(nc, in_maps, core_ids=list(range(N_CORES)))
    out = np.concatenate([res.results[c]["ys"] for c in range(N_CORES)], axis=0)
    return out.astype(np.float32)


def _make_in_maps(x: np.ndarray):
    xp = _prep_inputs(np.ascontiguousarray(x, dtype=np.float32))
    return [
        {
            "xs": xp[c * B_CORE : (c + 1) * B_CORE],
            "wred": _WRED,
            "dftc": _DFTC,
            "wrep": _WREP,
        }
        for c in range(N_CORES)
    ]


# revision 61
# speedup vs baseline: 1.0512x; 1.0512x over previous
"""Trainium2 Bass kernel for nn_DCTHighPass.

Reference computation (per sample, 512x512 RGB image):
  gray = 0.299 R + 0.587 G + 0.114 B
  tiles = 8x8 blocks of gray (64x64 tiles, row-major (ti, tj))
  mag = |fft2(tile)|
  (buggy mask touches only batch 3:6 / fft rows 3:6 -> never sampled below)
  img = mag tiles stacked into [4096*8, 8]
  out = bilinear_resize(img, 512, 512)

Key reduction: the height downsample (32768 -> 512, factor 64) samples only
input rows 64i+31 and 64i+32 with weight 0.5 each, i.e. fft-row 7 of tile
(ti=i//8, tj=8*(i%8)+3) and fft-row 0 of tile (ti, tj=8*(i%8)+4).  fft rows
0/7 of an 8x8 real tile need only three 8-weight row reductions of the tile
(plain sum, cos, sin), followed by an 8-point DFT along columns.  The width
upsample (8 -> 512) is a fixed [8,512] matrix.  So per output row i:
  v = 0.5*(|DFT(cos/sin rowsums of tile tj3)| + |DFT(colsum of tile tj4)|)
  out[i, :] = v @ W8
Only image columns 64p+24 .. 64p+39 (p = i%8) are ever used.

Layout strategy (the v1 kernel was DMA-bound at 75% on strided 64B
descriptors; this version is ~2.8x faster):
  - host gathers the 128 needed columns into a dense [B, 128p, 3ch*4q*128c]
    bf16 array, so device loads are contiguous 3KB-per-partition DMAs
    (strided 64B descriptors ran at ~9B/ns/engine vs 22.5 peak);
  - all matmuls and constants run in bf16 (fp32 pays 4 cycles/row on PE,
    bf16 pays 1; PSUM still accumulates fp32);
  - output is written bf16 (halves store traffic), merged 4KB-contiguous
    stores; host upcasts to fp32 (rel err ~0.5% vs the 2e-2 gate);
  - all 4 sample-pairs are emitted interleaved stage-by-stage
    (CFG interleave=4) so the Tile scheduler overlaps pairs.

Pipeline per 2 samples (per core, batch of 8 samples):
  load   xin [128, 1536] bf16 per sample (gpsimd queue)
  stage1 (PE): per (smp, q, ch): x chunk stationary, gray-coef-scaled
         wred [128,48] moving, ch-accumulated in PSUM
         -> ps1 [128=(p,cc), (q: type,tI_l)] -> rhs2 bf16 (DVE copy)
  stage2 (PE): 6 block-diag DFT matmuls -> psQ [64=(p,k), 512]
  magnitude (ACT/DVE): sq -> pair-sum -> 0.5*sqrt -> vt [64, 128] bf16
  stage3 (PE): vt stationary @ p-masked replicated W8 -> 8x [128, 512]
  drain PSUM -> out_all [128=(smp,tI), 8p*512j] bf16 (DVE/ACT copies)
  store ys rows 8*tI+p: two [64, 2048] DMAs per sample (sync queue)
"""

import sys

sys.path.insert(0, "/opt/trn_rl_repo")

import numpy as np
import ml_dtypes

from concourse import bacc
import concourse.mybir as mybir
from concourse.tile import TileContext
from concourse.bass_utils import run_bass_kernel_spmd

N_CORES = 8
B_FULL = 64
B_CORE = B_FULL // N_CORES  # 8 samples per core
H = W = 512
K = 8  # fft tile size
NQ = 4  # 128-row chunks per image
F32 = mybir.dt.float32
BF16 = mybir.dt.bfloat16
F32R = mybir.dt.float32r

# image columns ever sampled by the width resize: 64p+24 .. 64p+39
_COLS = np.concatenate([np.arange(64 * p + 24, 64 * p + 40) for p in range(K)])


# ----------------------------------------------------------------------------
# host-side constants
# ----------------------------------------------------------------------------
def _make_constants():
    j = np.arange(K)
    cosr = np.cos(2 * np.pi * j / K)
    sinr = np.sin(2 * np.pi * j / K)

    # wred [128, 144]: free = 48*ch + 16*type + tI_l, gray coef folded in;
    # type 0: plain sum (A), 1: cos rowsum (Cr), 2: sin rowsum (Ci);
    # partition = 8*tI_l + row
    coef = [0.299, 0.587, 0.114]
    wtypes = [np.ones(K), cosr, sinr]
    wred = np.zeros((128, 144), dtype=np.float32)
    for ch in range(3):
        for ty in range(3):
            for t in range(16):
                wred[8 * t : 8 * t + 8, 48 * ch + 16 * ty + t] = (
                    coef[ch] * wtypes[ty]
                )

    # dft matrices C[v,c] = cos(2pi v c/8), S[v,c] = sin(2pi v c/8)
    v = np.arange(K)
    C8 = np.cos(2 * np.pi * np.outer(v, j) / K).astype(np.float32)
    S8 = np.sin(2 * np.pi * np.outer(v, j) / K).astype(np.float32)

    # dftc [128, 640]: five zero-padded [128, 128] stationaries whose out
    # cols map to 128 partitions (g34, p, k): g0 -> 8p+k, g1 -> 64+8p+k.
    # blocks: 0 [C|0], 1 [S|0], 2 [0|C], 3 [-S|0], 4 [0|S]
    # Re chain: b0@selCr + b1@selCi + b2@selA
    # Im chain: b0@selCi + b3@selCr + b4@selA
    # input partition = 16p + cc (cc in 0..15, g = cc//8)
    dftc = np.zeros((128, 640), dtype=np.float32)
    for p in range(8):
        for cc in range(16):
            g, c = divmod(cc, 8)
            for k in range(8):
                cv, sv = C8[k, c], S8[k, c]
                if g == 0:
                    dftc[16 * p + cc, 0 * 128 + 8 * p + k] = cv
                    dftc[16 * p + cc, 1 * 128 + 8 * p + k] = sv
                    dftc[16 * p + cc, 3 * 128 + 8 * p + k] = -sv
                else:
                    dftc[16 * p + cc, 2 * 128 + 64 + 8 * p + k] = cv
                    dftc[16 * p + cc, 4 * 128 + 64 + 8 * p + k] = sv

    # W8 [8, 512]: bilinear width resize 8 -> 512 (align_corners=False)
    src = (np.arange(W) + 0.5) * (K / W) - 0.5
    src = np.clip(src, 0.0, K - 1.0)
    i0 = np.floor(src).astype(np.int64)
    i1 = np.minimum(i0 + 1, K - 1)
    fr = (src - i0).astype(np.float32)
    W8 = np.zeros((K, W), dtype=np.float32)
    for jj in range(W):
        W8[i0[jj], jj] += 1.0 - fr[jj]
        W8[i1[jj], jj] += fr[jj]

    # wrep [128, 8*512]: block p holds W8 on partitions 8p..8p+7 (group 3)
    # and 64+8p..64+8p+7 (group 4) — the stage-3 contraction over the
    # doubled partition dim performs the m3+m4 height-blend sum
    wrep = np.zeros((128, 8 * W), dtype=np.float32)
    for p in range(8):
        wrep[8 * p : 8 * p + 8, W * p : W * p + W] = W8
        wrep[64 + 8 * p : 64 + 8 * p + 8, W * p : W * p + W] = W8

    return (
        wred.astype(ml_dtypes.bfloat16),
        dftc.astype(ml_dtypes.bfloat16),
        wrep.astype(ml_dtypes.bfloat16),
    )


_WRED, _DFTC, _WREP = _make_constants()


# ----------------------------------------------------------------------------
# bass program (identical on all cores; per-core inputs differ)
# ----------------------------------------------------------------------------
CFG = dict(xin_bufs=4, mid_bufs=5, out_bufs=3, ps1_bufs=3, ps2_bufs=2,
           ps3_bufs=3, copy_pat="avavavav", split_load=1, split_store=2, interleave=4, direct_store=False, load_q="gpsimd", sq_on_dve=False, store_split_q="smp", s2_reorder=False)


def _build_program(repeat=1, variant="full", unroll=False):
    nold = variant in ("nold", "nodma")
    nost = variant in ("nost", "nodma")
    nc = bacc.Bacc()

    xs = nc.declare_dram_parameter("xs", [B_CORE, 128, 3 * NQ * 128], BF16, isOutput=False)
    wred_d = nc.declare_dram_parameter("wred", [128, 144], BF16, isOutput=False)
    dftc_d = nc.declare_dram_parameter("dftc", [128, 640], BF16, isOutput=False)
    wrep_d = nc.declare_dram_parameter("wrep", [128, 8 * W], BF16, isOutput=False)
    ys = nc.declare_dram_parameter("ys", [B_CORE, 1, H, W], BF16, isOutput=True)

    with TileContext(nc) as tc:
        with (
            tc.tile_pool(name="consts", bufs=1) as cpool,
            tc.tile_pool(name="xin", bufs=CFG["xin_bufs"]) as xpool,
            tc.tile_pool(name="mid", bufs=CFG["mid_bufs"]) as mpool,
            tc.tile_pool(name="outp", bufs=CFG["out_bufs"]) as opool,
            tc.tile_pool(name="ps1", bufs=CFG["ps1_bufs"], space="PSUM") as ps1pool,
            tc.tile_pool(name="ps2", bufs=CFG["ps2_bufs"], space="PSUM") as ps2pool,
            tc.tile_pool(name="ps3", bufs=CFG["ps3_bufs"], space="PSUM") as ps3pool,
        ):
            wred_sb = cpool.tile([128, 144], BF16, tag="wred")
            nc.sync.dma_start(wred_sb[:], wred_d[:])
            dftc_sb = cpool.tile([128, 640], BF16, tag="dftc")
            nc.sync.dma_start(dftc_sb[:], dftc_d[:])
            wrep_sb = cpool.tile([128, 8 * W], BF16, tag="wrep")
            nc.scalar.dma_start(wrep_sb[:], wrep_d[:])
            xconst = []
            if nold:
                # ablation: inputs loaded once, loop reads static tiles
                for smp in range(2):
                    xc = cpool.tile([128, 3 * NQ * 128], BF16, tag=f"xc{smp}")
                    nc.gpsimd.dma_start(xc[:], xs[smp])
                    xconst.append(
                        xc.rearrange("p (ch q c) -> p ch q c", ch=3, q=NQ)
                    )

            B0 = dftc_sb[:, 0:128]    # [C|0]
            B1 = dftc_sb[:, 128:256]  # [S|0]
            B2 = dftc_sb[:, 256:384]  # [0|C]
            B3 = dftc_sb[:, 384:512]  # [-S|0]
            B4 = dftc_sb[:, 512:640]  # [0|S]

            def do_loads(bg2):
                # per sample: one contiguous [128, 1536] bf16 DMA, or one
                # [128, 512] DMA per channel into its own tile (ch_tiles)
                # so stage-1 starts after the first channel lands
                if nold:
                    return lambda smp, ch, q: xconst[smp][:, ch, q]
                xn = []
                for smp in range(2):
                    bg = 2 * bg2 + smp
                    if CFG.get("load_split_q") == "sy" and bg2 % 2 == 1:
                        # odd pairs load via the sync queue: its sequencer is
                        # idle until the (late) stores, so the two queues
                        # halve the serial load phase at the body head
                        ldq = nc.sync
                    else:
                        ldq = getattr(nc, CFG["load_q"])
                    if CFG.get("ch_tiles"):
                        xsv = xs[bg].rearrange("p (ch r) -> p ch r", ch=3)
                        chts = []
                        for ch in range(3):
                            t = xpool.tile(
                                [128, NQ * 128], BF16, tag=f"xc{smp}{ch}"
                            )
                            ldq.dma_start(t[:], xsv[:, ch])
                            chts.append(t.rearrange("p (q c) -> p q c", q=NQ))
                        xn.append(chts)
                    else:
                        xin = xpool.tile(
                            [128, 3 * NQ * 128], BF16, tag=f"xn{smp}"
                        )
                        ldq.dma_start(xin[:], xs[bg])
                        xn.append(
                            xin.rearrange("p (ch q c) -> p ch q c", ch=3, q=NQ)
                        )
                if CFG.get("ch_tiles"):
                    return lambda smp, ch, q: xn[smp][ch][:, q]
                return lambda smp, ch, q: xn[smp][:, ch, q]

            def do_stage1(xn):
                # gray folded into channel-accumulated row reductions
                rhs2 = mpool.tile([128, 2 * 192], BF16, tag="rhs2")
                for smp in range(2):
                    ps1 = ps1pool.tile([128, 192], F32, tag="ps1")
                    for q in range(NQ):
                        for ch in range(3):
                            lhs = (
                                xn(0, 0, 0)
                                if variant == "ld1"
                                else xn(smp, ch, q)
                            )
                            nc.tensor.matmul(
                                ps1[:, 48 * q : 48 * q + 48],
                                lhs,
                                wred_sb[:, 48 * ch : 48 * ch + 48],
                                start=(ch == 0), stop=(ch == 2),
                            )
                    nc.vector.tensor_copy(
                        rhs2[:, 192 * smp : 192 * smp + 192], ps1[:]
                    )
                return rhs2

            def do_stage2(rhs2):
                # DFT + height-blend fused via PSUM accumulation; groups 3/4
                # land on separate PARTITION halves via zero-padded lhsT:
                # psQ [128=(g34,p,k), 256] = [Re | Im] of (smp,q,tI_l)
                rhs2v = rhs2.rearrange("p (s q blk) -> p s q blk", s=2, q=NQ)
                selA = rhs2v[:, :, :, 0:16]
                selCr = rhs2v[:, :, :, 16:32]
                selCi = rhs2v[:, :, :, 32:48]
                psQ = ps2pool.tile([128, 256], F32, tag="psQ")
                nc.tensor.matmul(psQ[:, 0:128], B0, selCr, start=True, stop=False)
                nc.tensor.matmul(psQ[:, 0:128], B1, selCi, start=False, stop=False)
                nc.tensor.matmul(psQ[:, 0:128], B2, selA, start=False, stop=True)
                nc.tensor.matmul(psQ[:, 128:256], B0, selCi, start=True, stop=False)
                nc.tensor.matmul(psQ[:, 128:256], B3, selCr, start=False, stop=False)
                nc.tensor.matmul(psQ[:, 128:256], B4, selA, start=False, stop=True)
                return psQ

            def do_mag(psQ):
                # m = 0.5*sqrt(re^2 + im^2) per (g,p,k); the m3+m4 sum is
                # folded into stage-3's doubled-partition contraction
                sq = mpool.tile([128, 256], F32, tag="sq")
                nc.scalar.activation(
                    sq[:], psQ[:], mybir.ActivationFunctionType.Square
                )
                s34 = mpool.tile([128, 128], F32, tag="s34")
                nc.vector.tensor_add(s34[:], sq[:, 0:128], sq[:, 128:256])
                vt = mpool.tile([128, 128], BF16, tag="vt")
                nc.scalar.activation(
                    vt[:], s34[:], mybir.ActivationFunctionType.Sqrt, scale=0.25
                )
                return vt

            def do_stage3(vt):
                # width resize; out partitions = (smp, tI)
                out_all = opool.tile([128, 8 * W], BF16, tag="out_all")
                wp = CFG.get("ps3_width", W)
                for i, p0 in enumerate(range(0, 8 * W, wp)):
                    ps3 = ps3pool.tile([128, wp], F32, tag="ps3")
                    nc.tensor.matmul(
                        ps3[:],
                        vt[:],
                        wrep_sb[:, p0 : p0 + wp],
                        start=True, stop=True,
                    )
                    dst = out_all[:, p0 : p0 + wp]
                    c = CFG["copy_pat"][i % len(CFG["copy_pat"])]
                    if c == "v":
                        nc.vector.tensor_copy(dst, ps3[:])
                    else:
                        nc.scalar.copy(dst, ps3[:])
                return out_all

            def do_stage3_direct(bg2, vt):
                # width resize with bf16 PSUM output, stored straight from
                # PSUM to HBM (no drain copies); out partitions = (smp, tI)
                if nost:
                    ysv = None
                else:
                    ysv = ys[2 * bg2 : 2 * bg2 + 2, 0].rearrange(
                        "s (t p) j -> (s t) (p j)", t=64
                    )
                for p in range(8):
                    ps3 = ps3pool.tile([128, W], BF16, tag="ps3")
                    nc.tensor.matmul(
                        ps3[:],
                        vt[:],
                        wrep_sb[:, W * p : W * p + W],
                        start=True, stop=True,
                    )
                    if ysv is not None:
                        nc.gpsimd.dma_start(ysv[:, W * p : W * p + W], ps3[:])

            def do_stores(bg2, out_all):
                # rows 8*tI + p are contiguous in (p j); split halves fire
                # as soon as their copies land
                if nost:
                    return
                ss = CFG["split_store"]
                for smp in range(2):
                    bg = 2 * bg2 + smp
                    dst = ys[bg, 0].rearrange(
                        "(t h p) j -> h t (p j)", t=64, h=ss
                    )
                    src = out_all[64 * smp : 64 * smp + 64, :].rearrange(
                        "t (h pj) -> t h pj", h=ss
                    )
                    for hh in range(ss):
                        mode = CFG.get("store_split_q")
                        if mode == "smp" and smp == 1:
                            stq = nc.gpsimd
                        elif mode == "hh" and hh % 2 == 1:
                            stq = nc.gpsimd
                        else:
                            stq = nc.sync
                        stq.dma_start(dst[hh], src[:, hh])

            rep_ctx = tc.For_i(0, repeat, 1) if repeat > 1 and not unroll else None
            if rep_ctx is not None:
                rep_ctx.__enter__()
            n_unroll = repeat if unroll else 1
            G = CFG["interleave"]
            NB = B_CORE // 2
            def emit_tail(b, vts):
                if CFG["direct_store"]:
                    do_stage3_direct(b, vts[b])
                else:
                    do_stores(b, do_stage3(vts[b]))

            skew = CFG.get("skew", 0)
            for u in range(n_unroll):
                if skew:
                    # software-pipelined emission: stage3/stores of pair
                    # b-skew follow stage1/2/mag of pair b, so stores spread
                    # across the body instead of bunching at the tail
                    xns = {b: do_loads(b) for b in range(NB)}
                    vts = {}
                    for b in range(NB):
                        vts[b] = do_mag(do_stage2(do_stage1(xns[b])))
                        if b >= skew:
                            emit_tail(b - skew, vts)
                    for b in range(NB - skew, NB):
                        emit_tail(b, vts)
                else:
                    for base in range(0, NB, G):
                        prs = list(range(base, min(base + G, NB)))
                        xns = {b: do_loads(b) for b in prs}
                        rhs = {b: do_stage1(xns[b]) for b in prs}
                        psq = {b: do_stage2(rhs[b]) for b in prs}
                        vts = {b: do_mag(psq[b]) for b in prs}
                        if CFG["direct_store"]:
                            for b in prs:
                                do_stage3_direct(b, vts[b])
                        else:
                            outs = {b: do_stage3(vts[b]) for b in prs}
                            for b in prs:
                                do_stores(b, outs[b])

            if rep_ctx is not None:
                rep_ctx.__exit__(None, None, None)

    nc.compile()
    return nc


_NC = None


def _get_program():
    global _NC
    if _NC is None:
        _NC = _build_program()
    return _NC


def _prep_inputs(x: np.ndarray) -> np.ndarray:
    """[64,3,512,512] f32 -> [64, 128, 1536] bf16 with the needed columns
    gathered and rows regrouped: out[s, p, (ch,q,c)] = x[s, ch, 128q+p, COLS[c]]."""
    xsel = x[:, :, :, _COLS]  # [64, 3, 512, 128]
    xr = xsel.reshape(B_FULL, 3, NQ, 128, 128).transpose(0, 3, 1, 2, 4)
    return np.ascontiguousarray(xr).reshape(B_FULL, 128, 3 * NQ * 128).astype(
        ml_dtypes.bfloat16
    )


def kernel(x: np.ndarray) -> np.ndarray:
    assert x.shape == (B_FULL, 3, H, W), x.shape
    x = np.ascontiguousarray(x, dtype=np.float32)
    xp = _prep_inputs(x)
    nc = _get_program()
    in_maps = []
    for c in range(N_CORES):
        in_maps.append(
            {
                "xs": xp[c * B_CORE : (c + 1) * B_CORE],
                "wred": _WRED,
                "dftc": _DFTC,
                "wrep": _WREP,
            }
        )
    res = run_bass_kernel_spmd(nc, in_maps, core_ids=list(range(N_CORES)))
    out = np.concatenate([res.results[c]["ys"] for c in range(N_CORES)], axis=0)
    return out.astype(np.float32)


def _make_in_maps(x: np.ndarray):
    xp = _prep_inputs(np.ascontiguousarray(x, dtype=np.float32))
    return [
        {
            "xs": xp[c * B_CORE : (c + 1) * B_CORE],
            "wred": _WRED,
            "dftc": _DFTC,
            "wrep": _WREP,
        }
        for c in range(N_CORES)
    ]
